# revision 8
# baseline (speedup 1.0000x reference)
"""GroupedSwiGLU MoE kernel for 8x Trainium2 NeuronCores.

Strategy: expert-parallel. Core e gets expert e's weights and its (padded)
token group. Inside each core:
  phase 1: gateT/upT[inter, tok] = Wg/Wu^T-contracted matmuls vs xT
  swiglu : hT = silu(gateT) * upT
  phase 2: out[tok, hid] = hT^T-contracted matmuls vs Wd, scaled by probs
All matmul operands bf16 (fp32 PSUM accumulate); host does the
transpose/tiling/padding and the final gather.
"""

import numpy as np
import ml_dtypes
from contextlib import ExitStack

import concourse.bass as bass
import concourse.mybir as mybir
import concourse.tile as tile
from concourse.bacc import Bacc
from concourse.bass_utils import run_bass_kernel_spmd

E = 8
HID = 2048
INTER = 1408
P = 128
KO_H = HID // P    # 16 k-tiles for phase-1 contraction
KO_I = INTER // P  # 11 k-tiles for phase-2 contraction / m-tiles in phase 1
TF = 512           # phase-1 moving free chunk (tokens)
NF = 512           # phase-2 moving free chunk (hid)

F32 = mybir.dt.float32
BF16 = mybir.dt.bfloat16
NP_BF16 = ml_dtypes.bfloat16

_nc_cache: dict = {}


def _build(T: int):
    """Per-core Bass program for T padded tokens (T % 512 == 0)."""
    nc = Bacc()
    xT = nc.dram_tensor("xT", [P, KO_H, T], BF16, kind="ExternalInput")
    wg = nc.dram_tensor("wg", [P, KO_I, KO_H, P], BF16, kind="ExternalInput")
    wu = nc.dram_tensor("wu", [P, KO_I, KO_H, P], BF16, kind="ExternalInput")
    wd = nc.dram_tensor("wd", [P, KO_I, HID], BF16, kind="ExternalInput")
    probs = nc.dram_tensor("probs", [P, T // P], F32, kind="ExternalInput")
    out = nc.dram_tensor("out", [T, HID], F32, kind="ExternalOutput")

    n_tf = T // TF
    n_t = T // P
    n_nf = HID // NF

    with tile.TileContext(nc) as tc, ExitStack() as ctx:
        resident = ctx.enter_context(tc.tile_pool(name="resident", bufs=1))
        wpool = ctx.enter_context(tc.tile_pool(name="weights", bufs=2))
        tmp = ctx.enter_context(tc.tile_pool(name="tmp", bufs=3))
        opool = ctx.enter_context(tc.tile_pool(name="outp", bufs=4))
        psum = ctx.enter_context(tc.tile_pool(name="psum", bufs=2, space="PSUM"))

        xT_sb = resident.tile([P, KO_H, T], BF16)
        for k in range(KO_H):
            nc.sync.dma_start(xT_sb[:, k], xT[:, k])
        wd_sb = resident.tile([P, KO_I, HID], BF16)
        for k in range(KO_I):
            nc.sync.dma_start(wd_sb[:, k], wd[:, k])
        probs_dma = resident.tile([P, T // P], F32)
        nc.sync.dma_start(probs_dma[:], probs[:])
        # Bounce through DVE so phase-2 scaling (DVE) only ever needs the PE
        # wait: the TensorScalar ISA slot can't carry a second (DMA) wait.
        probs_sb = resident.tile([P, T // P], F32)
        nc.vector.tensor_copy(probs_sb[:], probs_dma[:])
        hT_sb = resident.tile([P, KO_I, T], BF16)

        # Phase 1: for each inter m-tile, gateT/upT psum then fused silu*mul
        for m in range(KO_I):
            wg_m = wpool.tile([P, KO_H, P], BF16, tag="wg")
            nc.gpsimd.dma_start(wg_m[:], wg[:, m])
            wu_m = wpool.tile([P, KO_H, P], BF16, tag="wu")
            nc.gpsimd.dma_start(wu_m[:], wu[:, m])
            for f in range(n_tf):
                pg = psum.tile([P, TF], F32, tag="pg")
                pu = psum.tile([P, TF], F32, tag="pu")
                for k in range(KO_H):
                    nc.tensor.matmul(
                        pg[:], wg_m[:, k], xT_sb[:, k, bass.ts(f, TF)],
                        start=(k == 0), stop=(k == KO_H - 1),
                    )
                for k in range(KO_H):
                    nc.tensor.matmul(
                        pu[:], wu_m[:, k], xT_sb[:, k, bass.ts(f, TF)],
                        start=(k == 0), stop=(k == KO_H - 1),
                    )
                sg = tmp.tile([P, TF], F32, tag="sg")
                nc.scalar.activation(
                    sg[:], pg[:], mybir.ActivationFunctionType.Silu
                )
                # ACT copy of up-psum so the DVE mul has a single-engine wait
                su = tmp.tile([P, TF], F32, tag="su")
                nc.scalar.copy(su[:], pu[:])
                nc.vector.tensor_mul(
                    hT_sb[:, m, bass.ts(f, TF)], sg[:], su[:]
                )

        # Phase 2: out tiles [128 tok, 512 hid], contract over inter
        for t in range(n_t):
            for n in range(n_nf):
                po = psum.tile([P, NF], F32, tag="po")
                for k in range(KO_I):
                    nc.tensor.matmul(
                        po[:], hT_sb[:, k, bass.ts(t, P)],
                        wd_sb[:, k, bass.ts(n, NF)],
                        start=(k == 0), stop=(k == KO_I - 1),
                    )
                ot = opool.tile([P, NF], F32, tag="ot")
                nc.vector.tensor_scalar_mul(ot[:], po[:], probs_sb[:, t : t + 1])
                nc.sync.dma_start(out[bass.ts(t, P), bass.ts(n, NF)], ot[:])
    nc.finalize()
    return nc


def _pack_core(x_pad, probs_pad, wg_e, wu_e, wd_e, T):
    """Host-side tiling into the DRAM layouts the kernel expects."""
    # xT[p, k, t] = x_pad[t, k*128+p]
    xT = np.ascontiguousarray(
        x_pad.T.reshape(KO_H, P, T).transpose(1, 0, 2)
    ).astype(NP_BF16)
    # wg[p, m, k, i] = w_gate[k*128+p, m*128+i]
    wgt = np.ascontiguousarray(
        wg_e.reshape(KO_H, P, KO_I, P).transpose(1, 2, 0, 3)
    ).astype(NP_BF16)
    wut = np.ascontiguousarray(
        wu_e.reshape(KO_H, P, KO_I, P).transpose(1, 2, 0, 3)
    ).astype(NP_BF16)
    # wd[p, k, h] = w_down[k*128+p, h]
    wdt = np.ascontiguousarray(
        wd_e.reshape(KO_I, P, HID).transpose(1, 0, 2)
    ).astype(NP_BF16)
    # probs[p, o] = probs_pad[o*128+p]
    pr = np.ascontiguousarray(probs_pad.reshape(T // P, P).T).astype(np.float32)
    return {"xT": xT, "wg": wgt, "wu": wut, "wd": wdt, "probs": pr}


def _run(inputs, trace=False):
    x = np.asarray(inputs["permuted_x"], np.float32)
    probs = np.asarray(inputs["permuted_probs"], np.float32)
    wg = np.asarray(inputs["w_gate"], np.float32)
    wu = np.asarray(inputs["w_up"], np.float32)
    wd = np.asarray(inputs["w_down"], np.float32)
    counts = np.asarray(inputs["tokens_per_expert"]).astype(np.int64)
    offs = np.concatenate([[0], np.cumsum(counts)])
    assert offs[-1] == x.shape[0]

    T = int(max(1, counts.max()))
    T = ((T + 511) // 512) * 512

    key = T
    if key not in _nc_cache:
        _nc_cache[key] = _build(T)
    nc = _nc_cache[key]

    in_maps = []
    for e in range(E):
        n = int(counts[e])
        s = int(offs[e])
        x_pad = np.zeros((T, HID), np.float32)
        x_pad[:n] = x[s : s + n]
        p_pad = np.zeros((T,), np.float32)
        p_pad[:n] = probs[s : s + n]
        in_maps.append(_pack_core(x_pad, p_pad, wg[e], wu[e], wd[e], T))

    res = run_bass_kernel_spmd(nc, in_maps, core_ids=list(range(E)), trace=trace)

    y = np.empty((x.shape[0], HID), np.float32)
    for e in range(E):
        n = int(counts[e])
        s = int(offs[e])
        y[s : s + n] = res.results[e]["out"][:n]
    return y, res


def kernel(**inputs) -> np.ndarray:
    y, _ = _run(inputs, trace=False)
    return y
